# revision 31
# baseline (speedup 1.0000x reference)
"""Causal self-attention (GQA + RoPE + QK-norm) Trainium2 Bass kernel.

Sharding over 8 cores: core c handles batch b = c//2 and head-half hf = c%2
(8 q-heads, 2 kv-heads per core). Each core computes a partial output
[T, C] through its wo column slice; the host sums the two partials per batch.

Per-core dataflow (bf16 matmuls, fp32 accumulation/softmax):
  phase 1 (per 128-row t-tile): QKV projections from resident xT tiles;
    sum-of-squares stats on the raw projections (RoPE preserves norms);
    rsqrt via exp(-0.5*ln(x)) so the whole kernel uses one ACT table set;
    RoPE via pair-swapped negative-stride APs with head-broadcast tables;
    DMA-transpose q/k into [d, t] layout (k duplicated on both halves so
    both heads of a partition-pair can slice it at matching offsets).
  phase 2 (per head x 512-col i-chunk): S^T = kT.T @ qT into paired PSUM
    banks, one merged exp per pair (scores bounded by +-8 after qk-norm,
    so no max subtraction), causal mask multiply only on diagonal 128-blocks,
    PV with a ones column appended to V giving y and the softmax denominator
    in one accumulation, per-partition reciprocal * scale, DMA-transpose yT.
  phase 3: out[t, e] = yT.T @ woT, DMA to DRAM straight from PSUM.
Emission is software-pipelined: projections run two chunks ahead and
o-projection one chunk behind attention, interleaved between heads.
"""

import sys

sys.path.insert(0, "/opt/trn_rl_repo")

from contextlib import ExitStack

import numpy as np
import ml_dtypes

import concourse.bass as bass
import concourse.bacc as bacc
import concourse.tile as tile
from concourse import mybir
from concourse.bass_utils import run_bass_kernel_spmd

BF16 = ml_dtypes.bfloat16
F32 = mybir.dt.float32
BF = mybir.dt.bfloat16

T = 2048
C = 1024
D = 64
HL = 8          # local q heads per core
KVL = 2         # local kv heads per core
TJ = HL * D     # 512 local q dims
KJ = KVL * D    # 128 local kv dims
NT = T // 128   # 16 t-tiles
NCT = C // 128  # 8 contraction tiles
EPS = 1e-6
ROPE_THETA = 10000.0
SCALE = 1.0 / np.sqrt(D)  # 0.125

MULT = mybir.AluOpType.mult
ADD = mybir.AluOpType.add
EXP = mybir.ActivationFunctionType.Exp
LN = mybir.ActivationFunctionType.Ln

# diagonal j-tile packing offsets inside the two diag s-tiles:
# tile A holds dj=0 (w 512) at 0, dj=1 (w 384) at 512, dj=3 (w 128) at 896;
# tile B holds dj=2 (w 256) at 0.
DIAG_OFF = {0: (0, 0), 1: (0, 512), 3: (0, 896), 2: (1, 0)}


def _rev_pairs(ap):
    """AP reading (x1, x0) for each adjacent pair (x0, x1) along the last dim."""
    s, n = ap.ap[-1]
    return bass.AP(tensor=ap.tensor, offset=ap.offset + s,
                   ap=[*ap.ap[:-1], [-s, 2]])


def _bcast(ap, nrep):
    """Insert a step-0 repeat dim after the partition dim."""
    return bass.AP(tensor=ap.tensor, offset=ap.offset,
                   ap=[ap.ap[0], [0, nrep], *ap.ap[1:]])


def build_nc():
    nc = bacc.Bacc("TRN2", target_bir_lowering=False)
    xT = nc.dram_tensor("xT", [C, T], BF, kind="ExternalInput")
    wqT = nc.dram_tensor("wqT", [C, TJ], BF, kind="ExternalInput")
    wkT = nc.dram_tensor("wkT", [C, KJ], BF, kind="ExternalInput")
    wvT = nc.dram_tensor("wvT", [C, KJ], BF, kind="ExternalInput")
    woT = nc.dram_tensor("woT", [TJ, C], BF, kind="ExternalInput")
    cosr = nc.dram_tensor("cosr", [T, D], F32, kind="ExternalInput")  # 8*cos, pair-repeated
    sinp = nc.dram_tensor("sinp", [T, D], F32, kind="ExternalInput")  # -+8*sin interleaved
    maskd = nc.dram_tensor("mask", [128, 128], BF, kind="ExternalInput")
    out_d = nc.dram_tensor("out", [T, C], F32, kind="ExternalOutput")
    if _DEBUG:
        dbg_q = nc.dram_tensor("dbg_q", [4, 128, T], BF, kind="ExternalOutput")
        dbg_k = nc.dram_tensor("dbg_k", [KVL, 128, T], BF, kind="ExternalOutput")
        dbg_v = nc.dram_tensor("dbg_v", [128, NT, KVL, 68], BF, kind="ExternalOutput")
        dbg_y = nc.dram_tensor("dbg_y", [4, 128, T], BF, kind="ExternalOutput")

    with tile.TileContext(nc) as tc, ExitStack() as ctx:
        const = ctx.enter_context(tc.tile_pool(name="const", bufs=1))
        x_sb = [const.tile([128, T], BF, tag=f"x{i}", name=f"x{i}") for i in range(NCT)]
        wq_sb = [const.tile([128, TJ], BF, tag=f"wq{i}", name=f"wq{i}") for i in range(NCT)]
        wk_sb = [const.tile([128, KJ], BF, tag=f"wk{i}", name=f"wk{i}") for i in range(NCT)]
        wv_sb = [const.tile([128, KJ], BF, tag=f"wv{i}", name=f"wv{i}") for i in range(NCT)]
        wo_sb = [const.tile([128, C], BF, tag=f"wo{i}", name=f"wo{i}") for i in range(4)]
        cos_sb = const.tile([128, NT, D], F32, tag="cos")
        sin_sb = const.tile([128, NT, D], F32, tag="sin")
        mask_sb = const.tile([128, 128], BF, tag="mask")
        qT_sb = [const.tile([128, T], BF, tag=f"qT{g}", name=f"qT{g}") for g in range(4)]
        kT_sb = [const.tile([128, T], BF, tag=f"kT{g}", name=f"kT{g}") for g in range(KVL)]
        v_sb = const.tile([128, NT, KVL, 68], BF, tag="v")
        yT_sb = [const.tile([128, T], BF, tag=f"yT{g}", name=f"yT{g}") for g in range(4)]
        magic_sb = const.tile([128, 2, HL + KVL], mybir.dt.uint32, tag="magic")

        # x tiles on the SP queue (critical path); weights/tables on the
        # ACT-issued queue so the two HWDGE queues load in parallel.
        for i in range(NCT):
            nc.sync.dma_start(out=x_sb[i][:], in_=xT[i * 128:(i + 1) * 128, :])
        for i in range(3):
            nc.scalar.dma_start(out=wq_sb[i][:], in_=wqT[i * 128:(i + 1) * 128, :])
        for i in range(NCT):
            nc.scalar.dma_start(out=wk_sb[i][:], in_=wkT[i * 128:(i + 1) * 128, :])
            nc.scalar.dma_start(out=wv_sb[i][:], in_=wvT[i * 128:(i + 1) * 128, :])
        nc.scalar.dma_start(out=cos_sb[:], in_=cosr.rearrange("(n p) d -> p n d", p=128))
        nc.scalar.dma_start(out=sin_sb[:], in_=sinp.rearrange("(n p) d -> p n d", p=128))
        nc.scalar.dma_start(out=mask_sb[:], in_=maskd[:])
        for i in range(3, NCT):
            nc.scalar.dma_start(out=wq_sb[i][:], in_=wqT[i * 128:(i + 1) * 128, :])
        for i in range(4):
            nc.scalar.dma_start(out=wo_sb[i][:], in_=woT[i * 128:(i + 1) * 128, :])
        nc.vector.memset(magic_sb[:], 0x5F3759DF)
        nc.vector.memset(v_sb[:], 0.0)
        nc.vector.memset(v_sb[:, :, :, 64:65], 1.0)

        # PSUM: s2 (2 banks) + s3 (3 banks) double-buffer the S->exp pipe;
        # q/kv/y/o share three 1-bank slots.
        s_pool = ctx.enter_context(tc.tile_pool(name="s_ps", bufs=2, space="PSUM"))
        y_pool = ctx.enter_context(tc.tile_pool(name="y_ps", bufs=1, space="PSUM"))
        pqo_pool = ctx.enter_context(tc.tile_pool(name="pqo_ps", bufs=1, space="PSUM"))
        o_pool = pqo_pool
        pps = pqo_pool
        st_pool = ctx.enter_context(tc.tile_pool(name="stats", bufs=3))
        qn_pool = ctx.enter_context(tc.tile_pool(name="qn", bufs=3))
        pt_pool = ctx.enter_context(tc.tile_pool(name="pt", bufs=10))
        y2_pool = ctx.enter_context(tc.tile_pool(name="y2", bufs=4))
        sm_pool = ctx.enter_context(tc.tile_pool(name="small", bufs=4))

        # Packing of each (head, chunk)'s S tiles into alternating 3-bank /
        # 2-bank PSUM tiles: "DA" = diagonal j-tiles 0,1,3 (widths 512/384/128
        # at +0/+512/+896), "D2" = diagonal j-tile 2 (width 256), ints = full
        # j-tile indices (width 512 each).
        S_PACKS = {
            0: [(["DA", "D2"], 1280)],
            1: [(["DA", "D2"], 1280), ([0, 1, 2], 1536), ([3], 512)],
            2: [(["DA", "D2"], 1280), ([0, 1, 2], 1536), ([3, 4, 5], 1536),
                ([6, 7], 1024)],
            3: [(["DA", "D2"], 1280), ([0, 1, 2], 1536), ([3, 4, 5], 1536),
                ([6, 7, 8], 1536), ([9, 10, 11], 1536)],
        }
        DA_SUB = {0: 0, 1: 512, 3: 896}

        pst = {}  # paired-stats state across even/odd proj tiles

        def proj_tile(tt):
            ts = slice(tt * 128, (tt + 1) * 128)
            cos_t = cos_sb[:, tt, :]
            sin_t = sin_sb[:, tt, :]
            sl = tt % 2  # stats slot

            q_ps = pps.tile([128, TJ], F32, tag="pqo", name="q_ps")
            for i in range(NCT):
                nc.tensor.matmul(q_ps[:], x_sb[i][:, ts], wq_sb[i][:],
                                 start=(i == 0), stop=(i == NCT - 1))
            kv_ps = pps.tile([128, 2, KJ], F32, tag="pqo", name="kv_ps")
            k_ps = kv_ps[:, 0, :]
            for i in range(NCT):
                nc.tensor.matmul(k_ps, x_sb[i][:, ts], wk_sb[i][:],
                                 start=(i == 0), stop=(i == NCT - 1))

            # rope on q (all heads at once via step-0 broadcast tables)
            rot = st_pool.tile([128, TJ], F32, tag=f"rot{sl}", name="rot")
            t1 = st_pool.tile([128, TJ], F32, tag="t1")
            q3 = q_ps[:].rearrange("p (h i two) -> p h i two", h=HL, two=2)
            nc.vector.tensor_mul(
                t1[:].rearrange("p (h i two) -> p h i two", h=HL, two=2),
                _rev_pairs(q3),
                _bcast(sin_t.rearrange("p (i two) -> p i two", two=2), HL))
            nc.vector.tensor_mul(
                rot[:].rearrange("p (h d) -> p h d", h=HL),
                q_ps[:].rearrange("p (h d) -> p h d", h=HL),
                _bcast(cos_t, HL))
            nc.vector.tensor_add(rot[:], rot[:], t1[:])

            # rope on k (both kv heads)
            rotk = st_pool.tile([128, KJ], F32, tag=f"rotk{sl}", name="rotk")
            t1k = st_pool.tile([128, KJ], F32, tag="t1k")
            k3 = k_ps.rearrange("p (h i two) -> p h i two", h=KVL, two=2)
            nc.vector.tensor_mul(
                t1k[:].rearrange("p (h i two) -> p h i two", h=KVL, two=2),
                _rev_pairs(k3),
                _bcast(sin_t.rearrange("p (i two) -> p i two", two=2), KVL))
            nc.vector.tensor_mul(
                rotk[:].rearrange("p (h d) -> p h d", h=KVL),
                k_ps.rearrange("p (h d) -> p h d", h=KVL),
                _bcast(cos_t, KVL))
            nc.vector.tensor_add(rotk[:], rotk[:], t1k[:])

            # sum-of-squares stats from the rotated values (RoPE preserves
            # norms; rot/rotk are in SBUF so DVE reads only one PSUM input)
            if sl == 0:
                pst["ssq2"] = st_pool.tile([128, 2, HL + KVL], F32, tag="ssq", name="ssq2")
            ssq = pst["ssq2"][:, sl, :]
            scr = st_pool.tile([128, TJ], F32, tag="scr")
            nc.vector.tensor_mul(scr[:], rot[:], rot[:])
            nc.vector.reduce_sum(ssq[:, 0:HL],
                                 scr[:].rearrange("p (h d) -> p h d", h=HL),
                                 axis=mybir.AxisListType.X)
            scrk = st_pool.tile([128, KJ], F32, tag="scrk")
            nc.vector.tensor_mul(scrk[:], rotk[:], rotk[:])
            nc.vector.reduce_sum(ssq[:, HL:HL + KVL],
                                 scrk[:].rearrange("p (h d) -> p h d", h=KVL),
                                 axis=mybir.AxisListType.X)

            v_ps = kv_ps[:, 1, :]
            for i in range(NCT):
                nc.tensor.matmul(v_ps, x_sb[i][:, ts], wv_sb[i][:],
                                 start=(i == 0), stop=(i == NCT - 1))
            for g in range(KVL):
                nc.vector.tensor_copy(v_sb[:, tt, g, 0:64],
                                      v_ps[:, g * D:(g + 1) * D])

            if sl == 0:
                pst["rot0"], pst["rotk0"] = rot, rotk
                return

            # finalize the pair: rinv = 1/sqrt(ssq + 64 eps) for both tiles,
            # computed on DVE (Newton from the bit-hack seed) so exp stays the
            # kernel's only ACT function (single table set).
            ns = [128, 2, HL + KVL]
            vv = st_pool.tile(ns, F32, tag="vv", name="vv")
            # stats came from the 8x-scaled rope output: ssq_rot = 64*ssq
            nc.vector.tensor_scalar(out=vv[:], in0=pst["ssq2"][:],
                                    scalar1=1.0 / 64.0, scalar2=64.0 * EPS,
                                    op0=MULT, op1=ADD)
            sh = st_pool.tile(ns, mybir.dt.uint32, tag="rsh", name="rsh")
            nc.vector.tensor_scalar(out=sh[:], in0=vv[:].bitcast(mybir.dt.uint32),
                                    scalar1=1, scalar2=None,
                                    op0=mybir.AluOpType.logical_shift_right)
            rinv2 = st_pool.tile(ns, F32, tag="rinv", name="rinv2")
            nc.vector.tensor_sub(rinv2[:].bitcast(mybir.dt.uint32), magic_sb[:],
                                 sh[:])
            y2t = st_pool.tile(ns, F32, tag="rsy2", name="rsy2")
            for _ in range(2):
                nc.vector.tensor_mul(y2t[:], rinv2[:], rinv2[:])
                nc.vector.tensor_mul(y2t[:], y2t[:], vv[:])
                nc.vector.tensor_scalar(out=y2t[:], in0=y2t[:], scalar1=-0.5,
                                        scalar2=1.5, op0=MULT, op1=ADD)
                nc.vector.tensor_mul(rinv2[:], rinv2[:], y2t[:])
            for psl, ptt, prot, protk in ((0, tt - 1, pst["rot0"], pst["rotk0"]),
                                          (1, tt, rot, rotk)):
                pts2 = slice(ptt * 128, (ptt + 1) * 128)
                rinv = rinv2[:, psl, :]
                qn = qn_pool.tile([128, TJ], BF, tag="qn")
                for h in range(HL):
                    nc.vector.tensor_scalar_mul(qn[:, h * D:(h + 1) * D],
                                                prot[:, h * D:(h + 1) * D],
                                                rinv[:, h:h + 1])
                for g in range(4):
                    eng = nc.sync if g % 2 == 0 else nc.scalar
                    eng.dma_start_transpose(qT_sb[g][:, pts2],
                                            qn[:, g * 128:(g + 1) * 128])
                kn = qn_pool.tile([128, 2 * KJ], BF, tag="kn")
                for g in range(KVL):
                    for off in (0, D):
                        nc.vector.tensor_scalar_mul(
                            kn[:, g * 128 + off:g * 128 + off + D],
                            protk[:, g * D:(g + 1) * D],
                            rinv[:, HL + g:HL + g + 1])
                for g in range(KVL):
                    nc.sync.dma_start_transpose(kT_sb[g][:, pts2],
                                                kn[:, g * 128:(g + 1) * 128])

        def attn_S(ci, h):
            """Emit S matmuls + exp + diag masks for one head; returns the
            P^T tile map for attn_PV."""
            g = h // 4          # local kv head
            g4 = h // 2         # qT group tile
            po = (h % 2) * 64   # partition offset of this head's d-rows
            cs = slice(ci * 512, (ci + 1) * 512)
            nfull = 4 * ci
            smap = {}           # jt -> (pt_tile, base_col, anchor_it)
            for items, used in S_PACKS[ci]:
                s_ps = s_pool.tile([128, 1536], F32, tag="s", name="s_ps")
                pt = pt_pool.tile([128, 1536], BF, tag="pt", name="pt")
                masks = []
                off = 0
                for item in items:
                    if item == "DA":
                        for dj in (0, 1, 3):
                            jt = nfull + dj
                            base = off + DA_SUB[dj]
                            for it in range(jt, nfull + 4):
                                lo = base + (it - jt) * 128
                                nc.tensor.matmul(
                                    s_ps[:, lo:lo + 128],
                                    kT_sb[g][po:po + D, jt * 128:(jt + 1) * 128],
                                    qT_sb[g4][po:po + D, it * 128:(it + 1) * 128],
                                    start=True, stop=True)
                            smap[jt] = (pt, base, jt)
                            masks.append((pt, base))
                        off += 1024
                    elif item == "D2":
                        jt = nfull + 2
                        for it in range(jt, nfull + 4):
                            lo = off + (it - jt) * 128
                            nc.tensor.matmul(
                                s_ps[:, lo:lo + 128],
                                kT_sb[g][po:po + D, jt * 128:(jt + 1) * 128],
                                qT_sb[g4][po:po + D, it * 128:(it + 1) * 128],
                                start=True, stop=True)
                        smap[jt] = (pt, off, jt)
                        masks.append((pt, off))
                        off += 256
                    else:
                        jt = item
                        nc.tensor.matmul(
                            s_ps[:, off:off + 512],
                            kT_sb[g][po:po + D, jt * 128:(jt + 1) * 128],
                            qT_sb[g4][po:po + D, cs], start=True, stop=True)
                        smap[jt] = (pt, off, nfull)
                        off += 512
                nc.scalar.activation(pt[:, 0:used], s_ps[:, 0:used], EXP,
                                     scale=SCALE)
                for mpt, mcol in masks:
                    nc.vector.tensor_mul(mpt[:, mcol:mcol + 128],
                                         mpt[:, mcol:mcol + 128], mask_sb[:])
            return smap

        def attn_PV(ci, h, smap):
            g = h // 4
            g4 = h // 2
            po = (h % 2) * 64
            nfull = 4 * ci
            y_ps = y_pool.tile([128, 4, 65], F32, tag="y", name="y_ps")
            y2s = []
            for iq in range(4):
                it = nfull + iq
                for jt in range(it + 1):
                    pt, base, anchor = smap[jt]
                    col = base + (it - anchor) * 128
                    nc.tensor.matmul(y_ps[:, iq, :], pt[:, col:col + 128],
                                     v_sb[:, jt, g, 0:65],
                                     start=(jt == 0), stop=(jt == it))
                rin = sm_pool.tile([128, 1], F32, tag="rin")
                nc.vector.reciprocal(rin[:], y_ps[:, iq, 64:65])
                if h % 2 == 0:
                    y2 = y2_pool.tile([128, 128], BF, tag=f"y2_{iq}", name="y2")
                    attn_PV.y2_cur[iq] = y2
                y2 = attn_PV.y2_cur[iq]
                nc.vector.tensor_scalar_mul(y2[:, po:po + D], y_ps[:, iq, 0:64],
                                            rin[:])
                y2s.append(y2)
            if h % 2 == 1:
                for iq in range(4):
                    it = nfull + iq
                    eng = nc.sync if iq % 2 == 0 else nc.scalar
                    eng.dma_start_transpose(
                        yT_sb[g4][:, it * 128:(it + 1) * 128], y2s[iq][:])

        attn_PV.y2_cur = [None] * 4

        def oproj_tile(tt, ec):
            ts = slice(tt * 128, (tt + 1) * 128)
            o_ps = o_pool.tile([128, 512], F32, tag="pqo", name="o_ps")
            for jg in range(4):
                nc.tensor.matmul(o_ps[:], yT_sb[jg][:, ts],
                                 wo_sb[jg][:, ec * 512:(ec + 1) * 512],
                                 start=(jg == 0), stop=(jg == 3))
            o_sb = qn_pool.tile([128, 512], F32, tag="osb", name="o_sb")
            nc.vector.tensor_copy(o_sb[:], o_ps[:])
            nc.sync.dma_start(out=out_d[ts, ec * 512:(ec + 1) * 512], in_=o_sb[:])

        # ---- software-pipelined emission ----
        # Heads are software-pipelined: S(h+1) is emitted before PV(h) so the
        # tensor engine fills the next head's S tiles while ACT drains the
        # previous head's exps. Projections run one chunk ahead (even heads),
        # the output projection one chunk behind (odd heads).
        for tt in range(4):
            proj_tile(tt)
        prev = None  # (ci, h, smap)
        for ci in range(4):
            for h in range(HL):
                smap = attn_S(ci, h)
                if prev is not None:
                    pci, ph, psmap = prev
                    attn_PV(pci, ph, psmap)
                    if ph % 2 == 0 and pci < 3:
                        proj_tile(4 * (pci + 1) + ph // 2)
                    if ph % 2 == 1 and pci >= 1:
                        ptt = 4 * (pci - 1) + ph // 2
                        oproj_tile(ptt, 0)
                        oproj_tile(ptt, 1)
                prev = (ci, h, smap)
        attn_PV(*prev[:2], prev[2])
        oproj_tile(11, 0)
        oproj_tile(11, 1)
        for tt in range(12, 16):
            oproj_tile(tt, 0)
            oproj_tile(tt, 1)
        if _DEBUG:
            for g in range(4):
                nc.sync.dma_start(out=dbg_q[g], in_=qT_sb[g][:])
                nc.sync.dma_start(out=dbg_y[g], in_=yT_sb[g][:])
            for g in range(KVL):
                nc.sync.dma_start(out=dbg_k[g], in_=kT_sb[g][:])
            nc.sync.dma_start(out=dbg_v[:], in_=v_sb[:])

    nc.compile()
    return nc


def make_tables():
    inv_freq = (ROPE_THETA ** (-np.arange(0, D, 2, dtype=np.float32) / D)).astype(np.float32)
    t = np.arange(T, dtype=np.float32)
    freqs = t[:, None] * inv_freq[None, :]
    cos = 8.0 * np.cos(freqs).astype(np.float32)
    sin = 8.0 * np.sin(freqs).astype(np.float32)
    cosr = np.repeat(cos, 2, axis=1).astype(np.float32)
    sinp = np.empty((T, D), np.float32)
    sinp[:, 0::2] = -sin
    sinp[:, 1::2] = sin
    mask = np.triu(np.ones((128, 128), np.float32)).astype(BF16)
    return cosr, sinp, mask


def make_in_maps(x, wq, wk, wv, wo):
    x = np.asarray(x, dtype=np.float32)
    wq = np.asarray(wq, dtype=np.float32)
    wk = np.asarray(wk, dtype=np.float32)
    wv = np.asarray(wv, dtype=np.float32)
    wo = np.asarray(wo, dtype=np.float32)
    cosr, sinp, mask = make_tables()
    in_maps = []
    for c in range(8):
        b, hf = c // 2, c % 2
        qs = slice(hf * TJ, (hf + 1) * TJ)
        ks = slice(hf * KJ, (hf + 1) * KJ)
        in_maps.append({
            "xT": np.ascontiguousarray(x[b].T).astype(BF16),
            "wqT": np.ascontiguousarray(wq[qs].T).astype(BF16),
            "wkT": np.ascontiguousarray(wk[ks].T).astype(BF16),
            "wvT": np.ascontiguousarray(wv[ks].T).astype(BF16),
            "woT": np.ascontiguousarray(wo[:, qs].T).astype(BF16),
            "cosr": cosr,
            "sinp": sinp,
            "mask": mask,
        })
    return in_maps


_DEBUG = False

_CACHE = {}


def kernel(x, wq, wk, wv, wo, _trace=False):
    if "nc" not in _CACHE:
        _CACHE["nc"] = build_nc()
    nc = _CACHE["nc"]
    in_maps = make_in_maps(x, wq, wk, wv, wo)
    res = run_bass_kernel_spmd(nc, in_maps, core_ids=list(range(8)), trace=_trace)
    out = np.empty((4, T, C), np.float32)
    for b in range(4):
        out[b] = res.results[2 * b]["out"] + res.results[2 * b + 1]["out"]
    if _trace:
        _CACHE["last_result"] = res
    return out
